# revision 13
# baseline (speedup 1.0000x reference)
"""Trainium2 Bass kernel for the per-species (MoE-routed) atom MLP net.

Computation (see reference):
  x: [B=2048, A=64, D=384] f32, species: [A] int32 in [0, S=4)
  4-layer per-species MLP 384->128->128->64->1 with gaussian act exp(-y^2)
  between layers, then sum over atoms -> out [B].

Strategy:
  - Data-parallel over B across 8 cores (B_c = 256), no collectives.
  - Host-side: repack x into [feature-on-partition, batch-on-free] layout so
    all device DMAs are fully contiguous; group atoms by species into "units"
    of up to 2 atoms (N = 256 * n_atoms <= 512 matmul free dim, fp32).
  - exp(-(y+b)^2) is computed in ONE ScalarE op via Derivative_Erf:
    d/dx erf(x) = (2/sqrt(pi)) * exp(-x^2); the 2/sqrt(pi) factor is folded
    into the next layer's weights on the host (sqrt(pi)/2 scaling).
  - Layer 3 matmuls accumulate all units into one [1, 512] PSUM bank; the
    final fold (cols b + cols 256+b, + sum of b3 biases) happens on host.
"""

import math

import numpy as np

import concourse.bass as bass
import concourse.mybir as mybir
import concourse.tile as tile
from concourse.bass_utils import run_bass_kernel_spmd
from concourse.vector_clock import ScopedClock

AF = mybir.ActivationFunctionType
F32 = mybir.dt.float32
F32R = mybir.dt.float32r

# Matmul input dtype: float32r streams 1 col/cycle on the PE (vs 4 for
# float32) at ~TF32 effective multiply precision, fp32 accumulation.
USE_F32R = True

B, A, D, S = 2048, 64, 384, 4
H1, H2, H3 = 128, 128, 64
NCORES = 8
BC = B // NCORES  # 256 batch per core
NCH = D // 128  # 3 k-chunks for layer 0
DMA_GROUP = 2  # units per x DMA (bigger transfers amortize DMA overhead)

# Set by test harness to collect a profile; kernel() stores exec_time_ns here.
PROFILE = False
LAST_EXEC_NS = None
LAST_RESULTS = None

# ---------------------------------------------------------------------------
# Walrus in this toolchain rejects >1 sync-wait per instruction ("Too many
# sync wait commands", CoreV3GenImpl setupSyncWait).  Tile's semaphore
# assignment freely attaches several waits to one instruction, so any real
# Tile kernel trips it.  Post-pass: hoist all but one wait onto injected
# NoOps on the same engine queue immediately before the instruction — the
# queue executes them in order, so the blocking semantics are identical.
# ---------------------------------------------------------------------------


def _split_multi_waits(nc):
    import bass_rust

    n_split = 0
    for blk in nc.main_func.blocks:
        insts = blk.instructions
        idx = 0
        while idx < len(insts):
            ins = insts[idx]
            si = ins.sync_info
            if si is not None and si.on_wait and len(si.on_wait) > 1:
                waits = list(si.on_wait)
                si.on_wait = [waits[-1]]
                for w in waits[:-1]:
                    nop = bass_rust.InstNoOp(
                        name=nc.get_next_instruction_name(), ins=[], outs=[]
                    )
                    nop.engine = ins.engine
                    nop.sync_info = mybir.SyncInfo(on_wait=[w], on_update=[])
                    nc.register_instruction(nop)
                    insts.insert(idx, nop)
                    idx += 1
                    n_split += 1
            idx += 1
    return n_split


def _build_units(species: np.ndarray):
    """Group atom indices by species into units of <=2 atoms.

    Singles go last so the final pipeline-drain chain is as short as
    possible (weights are reloaded per matmul either way, so unit order is
    free).
    """
    units = []  # list of (species, [atom, ...])
    singles = []
    for s in range(S):
        atoms = [int(a) for a in np.nonzero(species == s)[0]]
        for i in range(0, len(atoms) - 1, 2):
            units.append((s, atoms[i : i + 2]))
        if len(atoms) % 2:
            singles.append((s, atoms[-1:]))
    return units + singles


# Weight blob column layout (one [128, WCOLS] f32 SBUF tile / DRAM tensor).
OFF_W0 = 0  # [s][c] at OFF_W0 + (s*NCH + c)*128, 128 cols, 128 parts
OFF_W1 = OFF_W0 + S * NCH * 128  # [s] at OFF_W1 + s*128, 128 cols
OFF_W2 = OFF_W1 + S * 128  # [s] at OFF_W2 + s*64, 64 cols
OFF_W3 = OFF_W2 + S * 64  # [s] at OFF_W3 + s, 1 col, 64 parts
OFF_B0 = OFF_W3 + S  # [s] at OFF_B0 + s, 1 col
OFF_B1 = OFF_B0 + S
OFF_B2 = OFF_B1 + S
WCOLS = OFF_B2 + S


def _pack_weights(W0, b0, W1, b1, W2, b2, W3, b3):
    c = math.sqrt(math.pi) / 2.0  # undo Derivative_Erf's 2/sqrt(pi)
    blob = np.zeros((128, WCOLS), np.float32)
    for s in range(S):
        for ch in range(NCH):
            blob[:, OFF_W0 + (s * NCH + ch) * 128 : OFF_W0 + (s * NCH + ch + 1) * 128] = (
                W0[s, ch * 128 : (ch + 1) * 128, :]
            )
        blob[:, OFF_W1 + s * 128 : OFF_W1 + (s + 1) * 128] = W1[s] * c
        blob[:, OFF_W2 + s * 64 : OFF_W2 + (s + 1) * 64] = W2[s] * c
        blob[:H3, OFF_W3 + s] = W3[s][:, 0] * c
        blob[:, OFF_B0 + s] = b0[s]
        blob[:, OFF_B1 + s] = b1[s]
        blob[:H3, OFF_B2 + s] = b2[s]
    return blob


def _pack_x(x, units):
    """Per-core flat x arrays.

    Per unit: block [128, NCH * w] where w = 256 * n_atoms; within chunk c the
    columns are (atom, b) so each layer-0 matmul rhs is [:, c*w:(c+1)*w].
    Returns (flat arrays per core, unit column offsets (in flat elems)).
    """
    # [A, D, B] so per (atom, chunk) the [128, BC] block is contiguous-ish
    xt = np.ascontiguousarray(x.transpose(1, 2, 0))  # [A, D, B]
    total = sum(128 * NCH * 256 * len(a) for _, a in units)
    per_core = []
    for core in range(NCORES):
        bsl = slice(core * BC, (core + 1) * BC)
        groups = []
        for gi in range(0, len(units), DMA_GROUP):
            # one [128, gcols] C-order block per DMA group
            cols = []
            for s, atoms in units[gi : gi + DMA_GROUP]:
                # [n_a, D, BC] -> [n_a, NCH, 128, BC] -> [128, NCH, n_a, BC]
                blk = xt[atoms, :, bsl].reshape(len(atoms), NCH, 128, BC)
                cols.append(blk.transpose(2, 1, 0, 3).reshape(128, -1))
            groups.append(np.hstack(cols).reshape(-1))
        flat = np.concatenate(groups)
        assert flat.size == total
        per_core.append(flat)
    return per_core, None, total


def _build_program(units, total_x, repeat=1):
    MDT = F32R if USE_F32R else F32
    nc = bass.Bass()
    xin = nc.dram_tensor("xin", [total_x], MDT, kind="ExternalInput")
    wts = nc.dram_tensor("wts", [128, WCOLS], MDT, kind="ExternalInput")
    xout = nc.dram_tensor("xout", [1, 512], F32, kind="ExternalOutput")

    with tile.TileContext(nc) as tc:
        with (
            tc.tile_pool(name="wpool", bufs=1) as wpool,
            tc.tile_pool(name="xpool", bufs=4) as xpool,
            tc.tile_pool(name="apool", bufs=3) as apool,
            tc.tile_pool(name="opool", bufs=1) as opool,
            tc.tile_pool(name="ypool", bufs=2, space="PSUM") as ypool,
            tc.tile_pool(name="outp", bufs=1, space="PSUM") as outp,
        ):
            def body():
                wt = wpool.tile([128, WCOLS], MDT, tag="wt")
                nc.sync.dma_start(wt[:], wts[:])

                out_ps = outp.tile([1, 512], F32, tag="ops")

                def w0_ap(s, ch):
                    o = OFF_W0 + (s * NCH + ch) * 128
                    return wt[:, o : o + 128]

                def bias_ap(off, s, p=128):
                    ap = wt[:p, off + s : off + s + 1]
                    return ap.bitcast(F32) if MDT is F32R else ap

                # DMA x in groups of DMA_GROUP units (bigger transfers)
                uoff = 0
                xtiles = {}
                for gi in range(0, len(units), DMA_GROUP):
                    grp = units[gi : gi + DMA_GROUP]
                    gcols = sum(NCH * 256 * len(a) for _, a in grp)
                    gt = xpool.tile([128, NCH * 512 * DMA_GROUP], MDT, tag="xu")
                    nc.sync.dma_start(
                        gt[:, :gcols],
                        xin[uoff : uoff + 128 * gcols].rearrange(
                            "(p n) -> p n", p=128
                        ),
                    )
                    uoff += 128 * gcols
                    co = 0
                    for j, (s, atoms) in enumerate(grp):
                        ucols = NCH * 256 * len(atoms)
                        xtiles[gi + j] = gt[:, co : co + ucols]
                        co += ucols

                for ui, (s, atoms) in enumerate(units):
                    w = 256 * len(atoms)
                    xt_u = xtiles[ui]

                    # ---- layer 0: [384 -> 128] over d-chunks, N = w
                    y0 = ypool.tile([128, 512], F32, tag="y0")
                    for ch in range(NCH):
                        nc.tensor.matmul(
                            y0[:, :w],
                            w0_ap(s, ch),
                            xt_u[:, ch * w : (ch + 1) * w],
                            start=(ch == 0),
                            stop=(ch == NCH - 1),
                        )
                    a0 = apool.tile([128, 512], MDT, tag="a0")
                    nc.scalar.activation(
                        a0[:, :w], y0[:, :w], AF.Derivative_Erf,
                        bias=bias_ap(OFF_B0, s),
                    )

                    # ---- layer 1: [128 -> 128]
                    y1 = ypool.tile([128, 512], F32, tag="y1")
                    nc.tensor.matmul(
                        y1[:, :w],
                        wt[:, OFF_W1 + s * 128 : OFF_W1 + (s + 1) * 128],
                        a0[:, :w], start=True, stop=True,
                    )
                    a1 = apool.tile([128, 512], MDT, tag="a1")
                    nc.scalar.activation(
                        a1[:, :w], y1[:, :w], AF.Derivative_Erf,
                        bias=bias_ap(OFF_B1, s),
                    )

                    # ---- layer 2: [128 -> 64]
                    y2 = ypool.tile([64, 512], F32, tag="y2")
                    nc.tensor.matmul(
                        y2[:, :w],
                        wt[:, OFF_W2 + s * 64 : OFF_W2 + (s + 1) * 64],
                        a1[:, :w], start=True, stop=True,
                    )
                    a2 = apool.tile([64, 512], MDT, tag="a2")
                    nc.scalar.activation(
                        a2[:, :w], y2[:, :w], AF.Derivative_Erf,
                        bias=bias_ap(OFF_B2, s, p=H3),
                    )

                    # ---- layer 3: [64 -> 1], accumulate over all units
                    nc.tensor.matmul(
                        out_ps[:, :w],
                        wt[:H3, OFF_W3 + s : OFF_W3 + s + 1],
                        a2[:, :w],
                        start=(ui == 0),
                        stop=(ui == len(units) - 1),
                    )

                ot = opool.tile([1, 512], F32, tag="ot")
                nc.vector.tensor_copy(ot[:], out_ps[:])
                nc.sync.dma_start(xout[:], ot[:])

            if repeat == 1:
                body()
            else:
                with tc.For_i(0, repeat, 1):
                    body()

    _split_multi_waits(nc)
    return nc


def _prep(x, species, W0, b0, W1, b1, W2, b2, W3, b3):
    x = np.asarray(x, np.float32)
    species = np.asarray(species)
    units = _build_units(species)
    blob = _pack_weights(
        np.asarray(W0, np.float32), np.asarray(b0, np.float32),
        np.asarray(W1, np.float32), np.asarray(b1, np.float32),
        np.asarray(W2, np.float32), np.asarray(b2, np.float32),
        np.asarray(W3, np.float32), np.asarray(b3, np.float32),
    )
    xs, _, total = _pack_x(x, units)
    b3sum = float(np.asarray(b3, np.float64)[species, 0].sum())
    in_maps = [{"xin": xs[c], "wts": blob} for c in range(NCORES)]
    return units, total, in_maps, b3sum


def kernel(x, species, W0, b0, W1, b1, W2, b2, W3, b3):
    global LAST_EXEC_NS, LAST_RESULTS
    units, total, in_maps, b3sum = _prep(
        x, species, W0, b0, W1, b1, W2, b2, W3, b3
    )
    nc = _build_program(units, total)
    res = run_bass_kernel_spmd(nc, in_maps, list(range(NCORES)))
    LAST_EXEC_NS = res.exec_time_ns
    LAST_RESULTS = res
    out = np.empty(B, np.float32)
    for c in range(NCORES):
        v = res.results[c]["xout"].reshape(512)
        out[c * BC : (c + 1) * BC] = (
            v[:256].astype(np.float64) + v[256:].astype(np.float64) + b3sum
        ).astype(np.float32)
    return out


def bench(x, species, W0, b0, W1, b1, W2, b2, W3, b3,
          reps=(256, 16384), tries=3):
    """Per-invocation HW time via on-device For_i loop slope.

    Runs the kernel body R times inside one NEFF for each R in reps and
    wall-clocks the execute call; the slope between the two R values
    cancels tunnel/upload overhead.  Includes ~2-3us/iter of Tile loop
    back-edge barrier cost (constant across kernel versions).
    """
    import time as _time

    units, total, in_maps, _ = _prep(
        x, species, W0, b0, W1, b1, W2, b2, W3, b3
    )
    cores = list(range(NCORES))
    timings = {}
    for R in reps:
        nc = _build_program(units, total, repeat=R)
        ts = []
        for _ in range(tries):
            t0 = _time.perf_counter()
            run_bass_kernel_spmd(nc, in_maps, cores)
            ts.append(_time.perf_counter() - t0)
        timings[R] = min(ts[1:]) if len(ts) > 1 else ts[0]
    r0, r1 = min(reps), max(reps)
    ns = (timings[r1] - timings[r0]) / (r1 - r0) * 1e9
    return ns, timings


# revision 14
# speedup vs baseline: 1.0099x; 1.0099x over previous
"""Trainium2 Bass kernel for the per-species (MoE-routed) atom MLP net.

Computation (see reference):
  x: [B=2048, A=64, D=384] f32, species: [A] int32 in [0, S=4)
  4-layer per-species MLP 384->128->128->64->1 with gaussian act exp(-y^2)
  between layers, then sum over atoms -> out [B].

Strategy:
  - Data-parallel over B across 8 cores (B_c = 256), no collectives.
  - Host-side: repack x into [feature-on-partition, batch-on-free] layout so
    all device DMAs are fully contiguous; group atoms by species into "units"
    of up to 2 atoms (N = 256 * n_atoms <= 512 matmul free dim, fp32).
  - exp(-(y+b)^2) is computed in ONE ScalarE op via Derivative_Erf:
    d/dx erf(x) = (2/sqrt(pi)) * exp(-x^2); the 2/sqrt(pi) factor is folded
    into the next layer's weights on the host (sqrt(pi)/2 scaling).
  - Layer 3 matmuls accumulate all units into one [1, 512] PSUM bank; the
    final fold (cols b + cols 256+b, + sum of b3 biases) happens on host.
"""

import math

import numpy as np

import concourse.bass as bass
import concourse.mybir as mybir
import concourse.tile as tile
from concourse.bass_utils import run_bass_kernel_spmd
from concourse.vector_clock import ScopedClock

AF = mybir.ActivationFunctionType
F32 = mybir.dt.float32
F32R = mybir.dt.float32r

# Matmul input dtype: float32r streams 1 col/cycle on the PE (vs 4 for
# float32) at ~TF32 effective multiply precision, fp32 accumulation.
USE_F32R = True

B, A, D, S = 2048, 64, 384, 4
H1, H2, H3 = 128, 128, 64
NCORES = 8
BC = B // NCORES  # 256 batch per core
NCH = D // 128  # 3 k-chunks for layer 0
DMA_GROUP = 1  # units per x DMA

# Set by test harness to collect a profile; kernel() stores exec_time_ns here.
PROFILE = False
LAST_EXEC_NS = None
LAST_RESULTS = None

# ---------------------------------------------------------------------------
# Walrus in this toolchain rejects >1 sync-wait per instruction ("Too many
# sync wait commands", CoreV3GenImpl setupSyncWait).  Tile's semaphore
# assignment freely attaches several waits to one instruction, so any real
# Tile kernel trips it.  Post-pass: hoist all but one wait onto injected
# NoOps on the same engine queue immediately before the instruction — the
# queue executes them in order, so the blocking semantics are identical.
# ---------------------------------------------------------------------------


def _split_multi_waits(nc):
    import bass_rust

    n_split = 0
    for blk in nc.main_func.blocks:
        insts = blk.instructions
        idx = 0
        while idx < len(insts):
            ins = insts[idx]
            si = ins.sync_info
            if si is not None and si.on_wait and len(si.on_wait) > 1:
                waits = list(si.on_wait)
                si.on_wait = [waits[-1]]
                for w in waits[:-1]:
                    nop = bass_rust.InstNoOp(
                        name=nc.get_next_instruction_name(), ins=[], outs=[]
                    )
                    nop.engine = ins.engine
                    nop.sync_info = mybir.SyncInfo(on_wait=[w], on_update=[])
                    nc.register_instruction(nop)
                    insts.insert(idx, nop)
                    idx += 1
                    n_split += 1
            idx += 1
    return n_split


def _build_units(species: np.ndarray):
    """Group atom indices by species into units of <=2 atoms.

    Singles go last so the final pipeline-drain chain is as short as
    possible (weights are reloaded per matmul either way, so unit order is
    free).
    """
    units = []  # list of (species, [atom, ...])
    singles = []
    for s in range(S):
        atoms = [int(a) for a in np.nonzero(species == s)[0]]
        for i in range(0, len(atoms) - 1, 2):
            units.append((s, atoms[i : i + 2]))
        if len(atoms) % 2:
            singles.append((s, atoms[-1:]))
    return units + singles


# Weight blob column layout (one [128, WCOLS] f32 SBUF tile / DRAM tensor).
OFF_W0 = 0  # [s][c] at OFF_W0 + (s*NCH + c)*128, 128 cols, 128 parts
OFF_W1 = OFF_W0 + S * NCH * 128  # [s] at OFF_W1 + s*128, 128 cols
OFF_W2 = OFF_W1 + S * 128  # [s] at OFF_W2 + s*64, 64 cols
OFF_W3 = OFF_W2 + S * 64  # [s] at OFF_W3 + s, 1 col, 64 parts
OFF_B0 = OFF_W3 + S  # [s] at OFF_B0 + s, 1 col
OFF_B1 = OFF_B0 + S
OFF_B2 = OFF_B1 + S
WCOLS = OFF_B2 + S


def _pack_weights(W0, b0, W1, b1, W2, b2, W3, b3):
    c = math.sqrt(math.pi) / 2.0  # undo Derivative_Erf's 2/sqrt(pi)
    blob = np.zeros((128, WCOLS), np.float32)
    for s in range(S):
        for ch in range(NCH):
            blob[:, OFF_W0 + (s * NCH + ch) * 128 : OFF_W0 + (s * NCH + ch + 1) * 128] = (
                W0[s, ch * 128 : (ch + 1) * 128, :]
            )
        blob[:, OFF_W1 + s * 128 : OFF_W1 + (s + 1) * 128] = W1[s] * c
        blob[:, OFF_W2 + s * 64 : OFF_W2 + (s + 1) * 64] = W2[s] * c
        blob[:H3, OFF_W3 + s] = W3[s][:, 0] * c
        blob[:, OFF_B0 + s] = b0[s]
        blob[:, OFF_B1 + s] = b1[s]
        blob[:H3, OFF_B2 + s] = b2[s]
    return blob


def _pack_x(x, units):
    """Per-core flat x arrays.

    Per unit: block [128, NCH * w] where w = 256 * n_atoms; within chunk c the
    columns are (atom, b) so each layer-0 matmul rhs is [:, c*w:(c+1)*w].
    Returns (flat arrays per core, unit column offsets (in flat elems)).
    """
    # [A, D, B] so per (atom, chunk) the [128, BC] block is contiguous-ish
    xt = np.ascontiguousarray(x.transpose(1, 2, 0))  # [A, D, B]
    total = sum(128 * NCH * 256 * len(a) for _, a in units)
    per_core = []
    for core in range(NCORES):
        bsl = slice(core * BC, (core + 1) * BC)
        groups = []
        for gi in range(0, len(units), DMA_GROUP):
            # one [128, gcols] C-order block per DMA group
            cols = []
            for s, atoms in units[gi : gi + DMA_GROUP]:
                # [n_a, D, BC] -> [n_a, NCH, 128, BC] -> [128, NCH, n_a, BC]
                blk = xt[atoms, :, bsl].reshape(len(atoms), NCH, 128, BC)
                cols.append(blk.transpose(2, 1, 0, 3).reshape(128, -1))
            groups.append(np.hstack(cols).reshape(-1))
        flat = np.concatenate(groups)
        assert flat.size == total
        per_core.append(flat)
    return per_core, None, total


def _build_program(units, total_x, repeat=1):
    MDT = F32R if USE_F32R else F32
    nc = bass.Bass()
    xin = nc.dram_tensor("xin", [total_x], MDT, kind="ExternalInput")
    wts = nc.dram_tensor("wts", [128, WCOLS], MDT, kind="ExternalInput")
    xout = nc.dram_tensor("xout", [1, 512], F32, kind="ExternalOutput")

    with tile.TileContext(nc) as tc:
        with (
            tc.tile_pool(name="wpool", bufs=1) as wpool,
            tc.tile_pool(name="xpool", bufs=6) as xpool,
            tc.tile_pool(name="apool", bufs=3) as apool,
            tc.tile_pool(name="opool", bufs=1) as opool,
            tc.tile_pool(name="ypool", bufs=2, space="PSUM") as ypool,
            tc.tile_pool(name="outp", bufs=1, space="PSUM") as outp,
        ):
            def body():
                wt = wpool.tile([128, WCOLS], MDT, tag="wt")
                nc.sync.dma_start(wt[:], wts[:])

                out_ps = outp.tile([1, 512], F32, tag="ops")

                def w0_ap(s, ch):
                    o = OFF_W0 + (s * NCH + ch) * 128
                    return wt[:, o : o + 128]

                def bias_ap(off, s, p=128):
                    ap = wt[:p, off + s : off + s + 1]
                    return ap.bitcast(F32) if MDT is F32R else ap

                # DMA x in groups of DMA_GROUP units (bigger transfers)
                uoff = 0
                xtiles = {}
                for gi in range(0, len(units), DMA_GROUP):
                    grp = units[gi : gi + DMA_GROUP]
                    gcols = sum(NCH * 256 * len(a) for _, a in grp)
                    gt = xpool.tile([128, NCH * 512 * DMA_GROUP], MDT, tag="xu")
                    nc.sync.dma_start(
                        gt[:, :gcols],
                        xin[uoff : uoff + 128 * gcols].rearrange(
                            "(p n) -> p n", p=128
                        ),
                    )
                    uoff += 128 * gcols
                    co = 0
                    for j, (s, atoms) in enumerate(grp):
                        ucols = NCH * 256 * len(atoms)
                        xtiles[gi + j] = gt[:, co : co + ucols]
                        co += ucols

                for ui, (s, atoms) in enumerate(units):
                    w = 256 * len(atoms)
                    xt_u = xtiles[ui]

                    # ---- layer 0: [384 -> 128] over d-chunks, N = w
                    y0 = ypool.tile([128, 512], F32, tag="y0")
                    for ch in range(NCH):
                        nc.tensor.matmul(
                            y0[:, :w],
                            w0_ap(s, ch),
                            xt_u[:, ch * w : (ch + 1) * w],
                            start=(ch == 0),
                            stop=(ch == NCH - 1),
                        )
                    a0 = apool.tile([128, 512], MDT, tag="a0")
                    nc.scalar.activation(
                        a0[:, :w], y0[:, :w], AF.Derivative_Erf,
                        bias=bias_ap(OFF_B0, s),
                    )

                    # ---- layer 1: [128 -> 128]
                    y1 = ypool.tile([128, 512], F32, tag="y1")
                    nc.tensor.matmul(
                        y1[:, :w],
                        wt[:, OFF_W1 + s * 128 : OFF_W1 + (s + 1) * 128],
                        a0[:, :w], start=True, stop=True,
                    )
                    a1 = apool.tile([128, 512], MDT, tag="a1")
                    nc.scalar.activation(
                        a1[:, :w], y1[:, :w], AF.Derivative_Erf,
                        bias=bias_ap(OFF_B1, s),
                    )

                    # ---- layer 2: [128 -> 64]
                    y2 = ypool.tile([64, 512], F32, tag="y2")
                    nc.tensor.matmul(
                        y2[:, :w],
                        wt[:, OFF_W2 + s * 64 : OFF_W2 + (s + 1) * 64],
                        a1[:, :w], start=True, stop=True,
                    )
                    a2 = apool.tile([64, 512], MDT, tag="a2")
                    nc.scalar.activation(
                        a2[:, :w], y2[:, :w], AF.Derivative_Erf,
                        bias=bias_ap(OFF_B2, s, p=H3),
                    )

                    # ---- layer 3: [64 -> 1], accumulate over all units
                    nc.tensor.matmul(
                        out_ps[:, :w],
                        wt[:H3, OFF_W3 + s : OFF_W3 + s + 1],
                        a2[:, :w],
                        start=(ui == 0),
                        stop=(ui == len(units) - 1),
                    )

                ot = opool.tile([1, 512], F32, tag="ot")
                nc.vector.tensor_copy(ot[:], out_ps[:])
                nc.sync.dma_start(xout[:], ot[:])

            if repeat == 1:
                body()
            else:
                with tc.For_i(0, repeat, 1):
                    body()

    _split_multi_waits(nc)
    return nc


def _prep(x, species, W0, b0, W1, b1, W2, b2, W3, b3):
    x = np.asarray(x, np.float32)
    species = np.asarray(species)
    units = _build_units(species)
    blob = _pack_weights(
        np.asarray(W0, np.float32), np.asarray(b0, np.float32),
        np.asarray(W1, np.float32), np.asarray(b1, np.float32),
        np.asarray(W2, np.float32), np.asarray(b2, np.float32),
        np.asarray(W3, np.float32), np.asarray(b3, np.float32),
    )
    xs, _, total = _pack_x(x, units)
    b3sum = float(np.asarray(b3, np.float64)[species, 0].sum())
    in_maps = [{"xin": xs[c], "wts": blob} for c in range(NCORES)]
    return units, total, in_maps, b3sum


def kernel(x, species, W0, b0, W1, b1, W2, b2, W3, b3):
    global LAST_EXEC_NS, LAST_RESULTS
    units, total, in_maps, b3sum = _prep(
        x, species, W0, b0, W1, b1, W2, b2, W3, b3
    )
    nc = _build_program(units, total)
    res = run_bass_kernel_spmd(nc, in_maps, list(range(NCORES)))
    LAST_EXEC_NS = res.exec_time_ns
    LAST_RESULTS = res
    out = np.empty(B, np.float32)
    for c in range(NCORES):
        v = res.results[c]["xout"].reshape(512)
        out[c * BC : (c + 1) * BC] = (
            v[:256].astype(np.float64) + v[256:].astype(np.float64) + b3sum
        ).astype(np.float32)
    return out


def bench(x, species, W0, b0, W1, b1, W2, b2, W3, b3,
          reps=(256, 16384), tries=3):
    """Per-invocation HW time via on-device For_i loop slope.

    Runs the kernel body R times inside one NEFF for each R in reps and
    wall-clocks the execute call; the slope between the two R values
    cancels tunnel/upload overhead.  Includes ~2-3us/iter of Tile loop
    back-edge barrier cost (constant across kernel versions).
    """
    import time as _time

    units, total, in_maps, _ = _prep(
        x, species, W0, b0, W1, b1, W2, b2, W3, b3
    )
    cores = list(range(NCORES))
    timings = {}
    for R in reps:
        nc = _build_program(units, total, repeat=R)
        ts = []
        for _ in range(tries):
            t0 = _time.perf_counter()
            run_bass_kernel_spmd(nc, in_maps, cores)
            ts.append(_time.perf_counter() - t0)
        timings[R] = min(ts[1:]) if len(ts) > 1 else ts[0]
    r0, r1 = min(reps), max(reps)
    ns = (timings[r1] - timings[r0]) / (r1 - r0) * 1e9
    return ns, timings


# revision 15
# speedup vs baseline: 1.1213x; 1.1103x over previous
"""Trainium2 Bass kernel for the per-species (MoE-routed) atom MLP net.

Computation (see reference):
  x: [B=2048, A=64, D=384] f32, species: [A] int32 in [0, S=4)
  4-layer per-species MLP 384->128->128->64->1 with gaussian act exp(-y^2)
  between layers, then sum over atoms -> out [B].

Strategy:
  - Data-parallel over B across 8 cores (B_c = 256), no collectives.
  - Host-side: repack x into [feature-on-partition, batch-on-free] layout so
    all device DMAs are fully contiguous; group atoms by species into "units"
    of up to 2 atoms (N = 256 * n_atoms <= 512 matmul free dim, fp32).
  - exp(-(y+b)^2) is computed in ONE ScalarE op via Derivative_Erf:
    d/dx erf(x) = (2/sqrt(pi)) * exp(-x^2); the 2/sqrt(pi) factor is folded
    into the next layer's weights on the host (sqrt(pi)/2 scaling).
  - Layer 3 matmuls accumulate all units into one [1, 512] PSUM bank; the
    final fold (cols b + cols 256+b, + sum of b3 biases) happens on host.
"""

import math

import numpy as np

import concourse.bass as bass
import concourse.mybir as mybir
import concourse.tile as tile
from concourse.bass_utils import run_bass_kernel_spmd
from concourse.vector_clock import ScopedClock

AF = mybir.ActivationFunctionType
F32 = mybir.dt.float32
F32R = mybir.dt.float32r

# Matmul input dtype: float32r streams 1 col/cycle on the PE (vs 4 for
# float32) at ~TF32 effective multiply precision, fp32 accumulation.
USE_F32R = True

B, A, D, S = 2048, 64, 384, 4
H1, H2, H3 = 128, 128, 64
NCORES = 8
BC = B // NCORES  # 256 batch per core
NCH = D // 128  # 3 k-chunks for layer 0
DMA_GROUP = 1  # units per x DMA

# Set by test harness to collect a profile; kernel() stores exec_time_ns here.
PROFILE = False
LAST_EXEC_NS = None
LAST_RESULTS = None

# ---------------------------------------------------------------------------
# Walrus in this toolchain rejects >1 sync-wait per instruction ("Too many
# sync wait commands", CoreV3GenImpl setupSyncWait).  Tile's semaphore
# assignment freely attaches several waits to one instruction, so any real
# Tile kernel trips it.  Post-pass: hoist all but one wait onto injected
# NoOps on the same engine queue immediately before the instruction — the
# queue executes them in order, so the blocking semantics are identical.
# ---------------------------------------------------------------------------


def _split_multi_waits(nc):
    import bass_rust

    n_split = 0
    for blk in nc.main_func.blocks:
        insts = blk.instructions
        idx = 0
        while idx < len(insts):
            ins = insts[idx]
            si = ins.sync_info
            if si is not None and si.on_wait and len(si.on_wait) > 1:
                waits = list(si.on_wait)
                si.on_wait = [waits[-1]]
                for w in waits[:-1]:
                    nop = bass_rust.InstNoOp(
                        name=nc.get_next_instruction_name(), ins=[], outs=[]
                    )
                    nop.engine = ins.engine
                    nop.sync_info = mybir.SyncInfo(on_wait=[w], on_update=[])
                    nc.register_instruction(nop)
                    insts.insert(idx, nop)
                    idx += 1
                    n_split += 1
            idx += 1
    return n_split


def _build_units(species: np.ndarray):
    """Group atom indices by species into units of <=2 atoms.

    Singles go last so the final pipeline-drain chain is as short as
    possible (weights are reloaded per matmul either way, so unit order is
    free).
    """
    units = []  # list of (species, [atom, ...])
    singles = []
    for s in range(S):
        atoms = [int(a) for a in np.nonzero(species == s)[0]]
        for i in range(0, len(atoms) - 1, 2):
            units.append((s, atoms[i : i + 2]))
        if len(atoms) % 2:
            singles.append((s, atoms[-1:]))
    return units + singles


# Weight blob column layout (one [128, WCOLS] f32 SBUF tile / DRAM tensor).
OFF_W0 = 0  # [s][c] at OFF_W0 + (s*NCH + c)*128, 128 cols, 128 parts
OFF_W1 = OFF_W0 + S * NCH * 128  # [s] at OFF_W1 + s*128, 128 cols
OFF_W2 = OFF_W1 + S * 128  # [s] at OFF_W2 + s*64, 64 cols
OFF_W3 = OFF_W2 + S * 64  # [s] at OFF_W3 + s, 1 col, 64 parts
OFF_B0 = OFF_W3 + S  # [s] at OFF_B0 + s, 1 col
OFF_B1 = OFF_B0 + S
OFF_B2 = OFF_B1 + S
WCOLS = OFF_B2 + S


def _pack_weights(W0, b0, W1, b1, W2, b2, W3, b3):
    c = math.sqrt(math.pi) / 2.0  # undo Derivative_Erf's 2/sqrt(pi)
    blob = np.zeros((128, WCOLS), np.float32)
    for s in range(S):
        for ch in range(NCH):
            blob[:, OFF_W0 + (s * NCH + ch) * 128 : OFF_W0 + (s * NCH + ch + 1) * 128] = (
                W0[s, ch * 128 : (ch + 1) * 128, :]
            )
        blob[:, OFF_W1 + s * 128 : OFF_W1 + (s + 1) * 128] = W1[s] * c
        blob[:, OFF_W2 + s * 64 : OFF_W2 + (s + 1) * 64] = W2[s] * c
        blob[:H3, OFF_W3 + s] = W3[s][:, 0] * c
        blob[:, OFF_B0 + s] = b0[s]
        blob[:, OFF_B1 + s] = b1[s]
        blob[:H3, OFF_B2 + s] = b2[s]
    return blob


def _pack_x(x, units):
    """Per-core flat x arrays.

    Per unit: block [128, NCH * w] where w = 256 * n_atoms; within chunk c the
    columns are (atom, b) so each layer-0 matmul rhs is [:, c*w:(c+1)*w].
    Returns (flat arrays per core, unit column offsets (in flat elems)).
    """
    # [A, D, B] so per (atom, chunk) the [128, BC] block is contiguous-ish
    xt = np.ascontiguousarray(x.transpose(1, 2, 0))  # [A, D, B]
    total = sum(128 * NCH * 256 * len(a) for _, a in units)
    per_core = []
    for core in range(NCORES):
        bsl = slice(core * BC, (core + 1) * BC)
        groups = []
        for gi in range(0, len(units), DMA_GROUP):
            # one [128, gcols] C-order block per DMA group
            cols = []
            for s, atoms in units[gi : gi + DMA_GROUP]:
                # [n_a, D, BC] -> [n_a, NCH, 128, BC] -> [128, NCH, n_a, BC]
                blk = xt[atoms, :, bsl].reshape(len(atoms), NCH, 128, BC)
                cols.append(blk.transpose(2, 1, 0, 3).reshape(128, -1))
            groups.append(np.hstack(cols).reshape(-1))
        flat = np.concatenate(groups)
        assert flat.size == total
        per_core.append(flat)
    return per_core, None, total


def _build_program(units, total_x, repeat=1):
    MDT = F32R if USE_F32R else F32
    nc = bass.Bass()
    xin = nc.dram_tensor("xin", [total_x], MDT, kind="ExternalInput")
    wts = nc.dram_tensor("wts", [128, WCOLS], MDT, kind="ExternalInput")
    xout = nc.dram_tensor("xout", [1, 512], F32, kind="ExternalOutput")

    with tile.TileContext(nc) as tc:
        with (
            tc.tile_pool(name="wpool", bufs=1) as wpool,
            tc.tile_pool(name="xpool", bufs=6) as xpool,
            tc.tile_pool(name="apool", bufs=3) as apool,
            tc.tile_pool(name="opool", bufs=1) as opool,
            tc.tile_pool(name="ypool", bufs=2, space="PSUM") as ypool,
            tc.tile_pool(name="outp", bufs=1, space="PSUM") as outp,
        ):
            def body():
                wt = wpool.tile([128, WCOLS], MDT, tag="wt")
                nc.sync.dma_start(wt[:], wts[:])

                out_ps = outp.tile([1, 512], F32, tag="ops")

                def w0_ap(s, ch):
                    o = OFF_W0 + (s * NCH + ch) * 128
                    return wt[:, o : o + 128]

                def bias_ap(off, s, p=128):
                    ap = wt[:p, off + s : off + s + 1]
                    return ap.bitcast(F32) if MDT is F32R else ap

                # DMA x in groups of DMA_GROUP units (bigger transfers)
                uoff = 0
                xtiles = {}
                for gi in range(0, len(units), DMA_GROUP):
                    grp = units[gi : gi + DMA_GROUP]
                    gcols = sum(NCH * 256 * len(a) for _, a in grp)
                    gt = xpool.tile([128, NCH * 512 * DMA_GROUP], MDT, tag="xu")
                    nc.sync.dma_start(
                        gt[:, :gcols],
                        xin[uoff : uoff + 128 * gcols].rearrange(
                            "(p n) -> p n", p=128
                        ),
                    )
                    uoff += 128 * gcols
                    co = 0
                    for j, (s, atoms) in enumerate(grp):
                        ucols = NCH * 256 * len(atoms)
                        xtiles[gi + j] = gt[:, co : co + ucols]
                        co += ucols

                for ui, (s, atoms) in enumerate(units):
                    w = 256 * len(atoms)
                    xt_u = xtiles[ui]

                    # ---- layer 0: [384 -> 128] over d-chunks, N = w
                    y0 = ypool.tile([128, 512], F32, tag="y0")
                    for ch in range(NCH):
                        nc.tensor.matmul(
                            y0[:, :w],
                            w0_ap(s, ch),
                            xt_u[:, ch * w : (ch + 1) * w],
                            start=(ch == 0),
                            stop=(ch == NCH - 1),
                        )
                    a0 = apool.tile([128, 512], MDT, tag="a0")
                    nc.scalar.activation(
                        a0[:, :w], y0[:, :w], AF.Derivative_Erf,
                        bias=bias_ap(OFF_B0, s),
                    )

                    # ---- layer 1: [128 -> 128]
                    y1 = ypool.tile([128, 512], F32, tag="y1")
                    nc.tensor.matmul(
                        y1[:, :w],
                        wt[:, OFF_W1 + s * 128 : OFF_W1 + (s + 1) * 128],
                        a0[:, :w], start=True, stop=True,
                    )
                    a1 = apool.tile([128, 512], MDT, tag="a1")
                    nc.scalar.activation(
                        a1[:, :w], y1[:, :w], AF.Derivative_Erf,
                        bias=bias_ap(OFF_B1, s),
                    )

                    # ---- layer 2: [128 -> 64]
                    y2 = ypool.tile([64, 512], F32, tag="y2")
                    nc.tensor.matmul(
                        y2[:, :w],
                        wt[:, OFF_W2 + s * 64 : OFF_W2 + (s + 1) * 64],
                        a1[:, :w], start=True, stop=True,
                    )
                    a2 = apool.tile([64, 512], MDT, tag="a2")
                    nc.scalar.activation(
                        a2[:, :w], y2[:, :w], AF.Derivative_Erf,
                        bias=bias_ap(OFF_B2, s, p=H3),
                    )

                    # ---- layer 3: [64 -> 1], accumulate over all units
                    nc.tensor.matmul(
                        out_ps[:, :w],
                        wt[:H3, OFF_W3 + s : OFF_W3 + s + 1],
                        a2[:, :w],
                        start=(ui == 0),
                        stop=(ui == len(units) - 1),
                    )

                ot = opool.tile([1, 512], F32, tag="ot")
                nc.vector.tensor_copy(ot[:], out_ps[:])
                nc.sync.dma_start(xout[:], ot[:])

            if repeat == 1:
                body()
            else:
                with tc.For_i(0, repeat, 1):
                    body()

    _split_multi_waits(nc)
    return nc


def _prep(x, species, W0, b0, W1, b1, W2, b2, W3, b3):
    x = np.asarray(x, np.float32)
    species = np.asarray(species)
    units = _build_units(species)
    blob = _pack_weights(
        np.asarray(W0, np.float32), np.asarray(b0, np.float32),
        np.asarray(W1, np.float32), np.asarray(b1, np.float32),
        np.asarray(W2, np.float32), np.asarray(b2, np.float32),
        np.asarray(W3, np.float32), np.asarray(b3, np.float32),
    )
    xs, _, total = _pack_x(x, units)
    b3sum = float(np.asarray(b3, np.float64)[species, 0].sum())
    in_maps = [{"xin": xs[c], "wts": blob} for c in range(NCORES)]
    return units, total, in_maps, b3sum


def kernel(x, species, W0, b0, W1, b1, W2, b2, W3, b3):
    global LAST_EXEC_NS, LAST_RESULTS
    units, total, in_maps, b3sum = _prep(
        x, species, W0, b0, W1, b1, W2, b2, W3, b3
    )
    nc = _build_program(units, total)
    res = run_bass_kernel_spmd(nc, in_maps, list(range(NCORES)))
    LAST_EXEC_NS = res.exec_time_ns
    LAST_RESULTS = res
    out = np.empty(B, np.float32)
    for c in range(NCORES):
        v = res.results[c]["xout"].reshape(512)
        out[c * BC : (c + 1) * BC] = (
            v[:256].astype(np.float64) + v[256:].astype(np.float64) + b3sum
        ).astype(np.float32)
    return out


def bench(x, species, W0, b0, W1, b1, W2, b2, W3, b3,
          reps=(256, 32768), tries=5):
    """Per-invocation HW time via on-device For_i loop slope.

    Runs the kernel body R times inside one NEFF for each R in reps and
    wall-clocks the execute call; the slope between the two R values
    cancels tunnel/upload overhead.  Includes ~2-3us/iter of Tile loop
    back-edge barrier cost (constant across kernel versions).
    """
    import time as _time

    units, total, in_maps, _ = _prep(
        x, species, W0, b0, W1, b1, W2, b2, W3, b3
    )
    cores = list(range(NCORES))
    timings = {}
    for R in reps:
        nc = _build_program(units, total, repeat=R)
        ts = []
        for _ in range(tries):
            t0 = _time.perf_counter()
            run_bass_kernel_spmd(nc, in_maps, cores)
            ts.append(_time.perf_counter() - t0)
        timings[R] = min(ts[1:]) if len(ts) > 1 else ts[0]
    r0, r1 = min(reps), max(reps)
    ns = (timings[r1] - timings[r0]) / (r1 - r0) * 1e9
    return ns, timings


# revision 16
# speedup vs baseline: 1.2786x; 1.1403x over previous
"""Trainium2 Bass kernel for the per-species (MoE-routed) atom MLP net.

Computation (see reference):
  x: [B=2048, A=64, D=384] f32, species: [A] int32 in [0, S=4)
  4-layer per-species MLP 384->128->128->64->1 with gaussian act exp(-y^2)
  between layers, then sum over atoms -> out [B].

Strategy:
  - Data-parallel over B across 8 cores (B_c = 256), no collectives.
  - Host-side: repack x into [feature-on-partition, batch-on-free] layout so
    all device DMAs are fully contiguous; group atoms by species into "units"
    of up to 2 atoms (N = 256 * n_atoms <= 512 matmul free dim, fp32).
  - exp(-(y+b)^2) is computed in ONE ScalarE op via Derivative_Erf:
    d/dx erf(x) = (2/sqrt(pi)) * exp(-x^2); the 2/sqrt(pi) factor is folded
    into the next layer's weights on the host (sqrt(pi)/2 scaling).
  - Layer 3 matmuls accumulate all units into one [1, 512] PSUM bank; the
    final fold (cols b + cols 256+b, + sum of b3 biases) happens on host.
"""

import math

import numpy as np

import concourse.bass as bass
import concourse.mybir as mybir
import concourse.tile as tile
from concourse.bass_utils import run_bass_kernel_spmd
from concourse.vector_clock import ScopedClock

AF = mybir.ActivationFunctionType
F32 = mybir.dt.float32
F32R = mybir.dt.float32r
F16 = mybir.dt.float16

# Matmul input dtype mode:
#   "f32"  — full fp32 (PE streams at 1/4 rate; exact, max rel ~8e-6)
#   "f32r" — fp32 storage, ~TF32 multiply precision, 1 col/cycle on PE
#   "f16"  — fp16 storage (half the DMA bytes), 1 col/cycle, eps 4.9e-4
MM_MODE = "f16"

B, A, D, S = 2048, 64, 384, 4
H1, H2, H3 = 128, 128, 64
NCORES = 8
BC = B // NCORES  # 256 batch per core
NCH = D // 128  # 3 k-chunks for layer 0
DMA_GROUP = 1  # units per x DMA

# Set by test harness to collect a profile; kernel() stores exec_time_ns here.
PROFILE = False
LAST_EXEC_NS = None
LAST_RESULTS = None

# ---------------------------------------------------------------------------
# Walrus in this toolchain rejects >1 sync-wait per instruction ("Too many
# sync wait commands", CoreV3GenImpl setupSyncWait).  Tile's semaphore
# assignment freely attaches several waits to one instruction, so any real
# Tile kernel trips it.  Post-pass: hoist all but one wait onto injected
# NoOps on the same engine queue immediately before the instruction — the
# queue executes them in order, so the blocking semantics are identical.
# ---------------------------------------------------------------------------


def _split_multi_waits(nc):
    import bass_rust

    n_split = 0
    for blk in nc.main_func.blocks:
        insts = blk.instructions
        idx = 0
        while idx < len(insts):
            ins = insts[idx]
            si = ins.sync_info
            if si is not None and si.on_wait and len(si.on_wait) > 1:
                waits = list(si.on_wait)
                si.on_wait = [waits[-1]]
                for w in waits[:-1]:
                    nop = bass_rust.InstNoOp(
                        name=nc.get_next_instruction_name(), ins=[], outs=[]
                    )
                    nop.engine = ins.engine
                    nop.sync_info = mybir.SyncInfo(on_wait=[w], on_update=[])
                    nc.register_instruction(nop)
                    insts.insert(idx, nop)
                    idx += 1
                    n_split += 1
            idx += 1
    return n_split


def _build_units(species: np.ndarray):
    """Group atom indices by species into units of <=2 atoms.

    Singles go last so the final pipeline-drain chain is as short as
    possible (weights are reloaded per matmul either way, so unit order is
    free).
    """
    units = []  # list of (species, [atom, ...])
    singles = []
    for s in range(S):
        atoms = [int(a) for a in np.nonzero(species == s)[0]]
        for i in range(0, len(atoms) - 1, 2):
            units.append((s, atoms[i : i + 2]))
        if len(atoms) % 2:
            singles.append((s, atoms[-1:]))
    return units + singles


# Weight blob column layout (one [128, WCOLS] f32 SBUF tile / DRAM tensor).
OFF_W0 = 0  # [s][c] at OFF_W0 + (s*NCH + c)*128, 128 cols, 128 parts
OFF_W1 = OFF_W0 + S * NCH * 128  # [s] at OFF_W1 + s*128, 128 cols
OFF_W2 = OFF_W1 + S * 128  # [s] at OFF_W2 + s*64, 64 cols
OFF_W3 = OFF_W2 + S * 64  # [s] at OFF_W3 + s, 1 col, 64 parts
OFF_B0 = OFF_W3 + S  # [s] at OFF_B0 + s, 1 col
OFF_B1 = OFF_B0 + S
OFF_B2 = OFF_B1 + S
WCOLS = OFF_B2 + S


def _pack_weights(W0, b0, W1, b1, W2, b2, W3, b3):
    c = math.sqrt(math.pi) / 2.0  # undo Derivative_Erf's 2/sqrt(pi)
    dt = np.float16 if MM_MODE == "f16" else np.float32
    blob = np.zeros((128, WCOLS), dt)
    for s in range(S):
        for ch in range(NCH):
            blob[:, OFF_W0 + (s * NCH + ch) * 128 : OFF_W0 + (s * NCH + ch + 1) * 128] = (
                W0[s, ch * 128 : (ch + 1) * 128, :]
            )
        blob[:, OFF_W1 + s * 128 : OFF_W1 + (s + 1) * 128] = W1[s] * c
        blob[:, OFF_W2 + s * 64 : OFF_W2 + (s + 1) * 64] = W2[s] * c
        blob[:H3, OFF_W3 + s] = W3[s][:, 0] * c
        blob[:, OFF_B0 + s] = b0[s]
        blob[:, OFF_B1 + s] = b1[s]
        blob[:H3, OFF_B2 + s] = b2[s]
    return blob


def _pack_x(x, units):
    """Per-core flat x arrays.

    Per unit: block [128, NCH * w] where w = 256 * n_atoms; within chunk c the
    columns are (atom, b) so each layer-0 matmul rhs is [:, c*w:(c+1)*w].
    Returns (flat arrays per core, unit column offsets (in flat elems)).
    """
    # [A, D, B] so per (atom, chunk) the [128, BC] block is contiguous-ish
    dt = np.float16 if MM_MODE == "f16" else np.float32
    xt = np.ascontiguousarray(x.transpose(1, 2, 0).astype(dt))  # [A, D, B]
    total = sum(128 * NCH * 256 * len(a) for _, a in units)
    per_core = []
    for core in range(NCORES):
        bsl = slice(core * BC, (core + 1) * BC)
        groups = []
        for gi in range(0, len(units), DMA_GROUP):
            # one [128, gcols] C-order block per DMA group
            cols = []
            for s, atoms in units[gi : gi + DMA_GROUP]:
                # [n_a, D, BC] -> [n_a, NCH, 128, BC] -> [128, NCH, n_a, BC]
                blk = xt[atoms, :, bsl].reshape(len(atoms), NCH, 128, BC)
                cols.append(blk.transpose(2, 1, 0, 3).reshape(128, -1))
            groups.append(np.hstack(cols).reshape(-1))
        flat = np.concatenate(groups)
        assert flat.size == total
        per_core.append(flat)
    return per_core, None, total


def _build_program(units, total_x, repeat=1):
    MDT = {"f32": F32, "f32r": F32R, "f16": F16}[MM_MODE]
    nc = bass.Bass()
    xin = nc.dram_tensor("xin", [total_x], MDT, kind="ExternalInput")
    wts = nc.dram_tensor("wts", [128, WCOLS], MDT, kind="ExternalInput")
    xout = nc.dram_tensor("xout", [1, 512], F32, kind="ExternalOutput")

    with tile.TileContext(nc) as tc:
        with (
            tc.tile_pool(name="wpool", bufs=1) as wpool,
            tc.tile_pool(name="xpool", bufs=6) as xpool,
            tc.tile_pool(name="apool", bufs=3) as apool,
            tc.tile_pool(name="opool", bufs=1) as opool,
            tc.tile_pool(name="ypool", bufs=2, space="PSUM") as ypool,
            tc.tile_pool(name="outp", bufs=1, space="PSUM") as outp,
        ):
            def body():
                wt = wpool.tile([128, WCOLS], MDT, tag="wt")
                nc.sync.dma_start(wt[:], wts[:])

                out_ps = outp.tile([1, 512], F32, tag="ops")

                def w0_ap(s, ch):
                    o = OFF_W0 + (s * NCH + ch) * 128
                    return wt[:, o : o + 128]

                def bias_ap(off, s, p=128):
                    ap = wt[:p, off + s : off + s + 1]
                    return ap.bitcast(F32) if MDT is F32R else ap

                # DMA x in groups of DMA_GROUP units (bigger transfers)
                uoff = 0
                xtiles = {}
                for gi in range(0, len(units), DMA_GROUP):
                    grp = units[gi : gi + DMA_GROUP]
                    gcols = sum(NCH * 256 * len(a) for _, a in grp)
                    gt = xpool.tile([128, NCH * 512 * DMA_GROUP], MDT, tag="xu")
                    nc.sync.dma_start(
                        gt[:, :gcols],
                        xin[uoff : uoff + 128 * gcols].rearrange(
                            "(p n) -> p n", p=128
                        ),
                    )
                    uoff += 128 * gcols
                    co = 0
                    for j, (s, atoms) in enumerate(grp):
                        ucols = NCH * 256 * len(atoms)
                        xtiles[gi + j] = gt[:, co : co + ucols]
                        co += ucols

                for ui, (s, atoms) in enumerate(units):
                    w = 256 * len(atoms)
                    xt_u = xtiles[ui]

                    # ---- layer 0: [384 -> 128] over d-chunks, N = w
                    y0 = ypool.tile([128, 512], F32, tag="y0")
                    for ch in range(NCH):
                        nc.tensor.matmul(
                            y0[:, :w],
                            w0_ap(s, ch),
                            xt_u[:, ch * w : (ch + 1) * w],
                            start=(ch == 0),
                            stop=(ch == NCH - 1),
                        )
                    a0 = apool.tile([128, 512], MDT, tag="a0")
                    nc.scalar.activation(
                        a0[:, :w], y0[:, :w], AF.Derivative_Erf,
                        bias=bias_ap(OFF_B0, s),
                    )

                    # ---- layer 1: [128 -> 128]
                    y1 = ypool.tile([128, 512], F32, tag="y1")
                    nc.tensor.matmul(
                        y1[:, :w],
                        wt[:, OFF_W1 + s * 128 : OFF_W1 + (s + 1) * 128],
                        a0[:, :w], start=True, stop=True,
                    )
                    a1 = apool.tile([128, 512], MDT, tag="a1")
                    nc.scalar.activation(
                        a1[:, :w], y1[:, :w], AF.Derivative_Erf,
                        bias=bias_ap(OFF_B1, s),
                    )

                    # ---- layer 2: [128 -> 64]
                    y2 = ypool.tile([64, 512], F32, tag="y2")
                    nc.tensor.matmul(
                        y2[:, :w],
                        wt[:, OFF_W2 + s * 64 : OFF_W2 + (s + 1) * 64],
                        a1[:, :w], start=True, stop=True,
                    )
                    a2 = apool.tile([64, 512], MDT, tag="a2")
                    nc.scalar.activation(
                        a2[:, :w], y2[:, :w], AF.Derivative_Erf,
                        bias=bias_ap(OFF_B2, s, p=H3),
                    )

                    # ---- layer 3: [64 -> 1], accumulate over all units
                    nc.tensor.matmul(
                        out_ps[:, :w],
                        wt[:H3, OFF_W3 + s : OFF_W3 + s + 1],
                        a2[:, :w],
                        start=(ui == 0),
                        stop=(ui == len(units) - 1),
                    )

                ot = opool.tile([1, 512], F32, tag="ot")
                nc.vector.tensor_copy(ot[:], out_ps[:])
                nc.sync.dma_start(xout[:], ot[:])

            if repeat == 1:
                body()
            else:
                with tc.For_i(0, repeat, 1):
                    body()

    _split_multi_waits(nc)
    return nc


def _prep(x, species, W0, b0, W1, b1, W2, b2, W3, b3):
    x = np.asarray(x, np.float32)
    species = np.asarray(species)
    units = _build_units(species)
    blob = _pack_weights(
        np.asarray(W0, np.float32), np.asarray(b0, np.float32),
        np.asarray(W1, np.float32), np.asarray(b1, np.float32),
        np.asarray(W2, np.float32), np.asarray(b2, np.float32),
        np.asarray(W3, np.float32), np.asarray(b3, np.float32),
    )
    xs, _, total = _pack_x(x, units)
    b3sum = float(np.asarray(b3, np.float64)[species, 0].sum())
    in_maps = [{"xin": xs[c], "wts": blob} for c in range(NCORES)]
    return units, total, in_maps, b3sum


def kernel(x, species, W0, b0, W1, b1, W2, b2, W3, b3):
    global LAST_EXEC_NS, LAST_RESULTS
    units, total, in_maps, b3sum = _prep(
        x, species, W0, b0, W1, b1, W2, b2, W3, b3
    )
    nc = _build_program(units, total)
    res = run_bass_kernel_spmd(nc, in_maps, list(range(NCORES)))
    LAST_EXEC_NS = res.exec_time_ns
    LAST_RESULTS = res
    out = np.empty(B, np.float32)
    for c in range(NCORES):
        v = res.results[c]["xout"].reshape(512)
        out[c * BC : (c + 1) * BC] = (
            v[:256].astype(np.float64) + v[256:].astype(np.float64) + b3sum
        ).astype(np.float32)
    return out


def bench(x, species, W0, b0, W1, b1, W2, b2, W3, b3,
          reps=(256, 32768), tries=5):
    """Per-invocation HW time via on-device For_i loop slope.

    Runs the kernel body R times inside one NEFF for each R in reps and
    wall-clocks the execute call; the slope between the two R values
    cancels tunnel/upload overhead.  Includes ~2-3us/iter of Tile loop
    back-edge barrier cost (constant across kernel versions).
    """
    import time as _time

    units, total, in_maps, _ = _prep(
        x, species, W0, b0, W1, b1, W2, b2, W3, b3
    )
    cores = list(range(NCORES))
    timings = {}
    for R in reps:
        nc = _build_program(units, total, repeat=R)
        ts = []
        for _ in range(tries):
            t0 = _time.perf_counter()
            run_bass_kernel_spmd(nc, in_maps, cores)
            ts.append(_time.perf_counter() - t0)
        timings[R] = min(ts[1:]) if len(ts) > 1 else ts[0]
    r0, r1 = min(reps), max(reps)
    ns = (timings[r1] - timings[r0]) / (r1 - r0) * 1e9
    return ns, timings


# revision 22
# speedup vs baseline: 1.6977x; 1.3278x over previous
"""Trainium2 Bass kernel for the per-species (MoE-routed) atom MLP net.

Computation (see reference):
  x: [B=2048, A=64, D=384] f32, species: [A] int32 in [0, S=4)
  4-layer per-species MLP 384->128->128->64->1 with gaussian act exp(-y^2)
  between layers, then sum over atoms -> out [B].

Strategy:
  - Data-parallel over B across 8 cores (B_c = 256), no collectives.
  - Host-side: repack x into [feature-on-partition, batch-on-free] layout so
    all device DMAs are fully contiguous; group atoms by species into "units"
    of up to 2 atoms (N = 256 * n_atoms <= 512 matmul free dim, fp32).
  - exp(-(y+b)^2) is computed in ONE ScalarE op via Derivative_Erf:
    d/dx erf(x) = (2/sqrt(pi)) * exp(-x^2); the 2/sqrt(pi) factor is folded
    into the next layer's weights on the host (sqrt(pi)/2 scaling).
  - Layer 3 matmuls accumulate all units into one [1, 512] PSUM bank; the
    final fold (cols b + cols 256+b, + sum of b3 biases) happens on host.
"""

import math

import numpy as np

import concourse.bass as bass
import concourse.mybir as mybir
import concourse.tile as tile
from concourse.bass_utils import run_bass_kernel_spmd
from concourse.vector_clock import ScopedClock

AF = mybir.ActivationFunctionType
F32 = mybir.dt.float32
F32R = mybir.dt.float32r
F16 = mybir.dt.float16

# Matmul input dtype mode:
#   "f32"  — full fp32 (PE streams at 1/4 rate; exact, max rel ~8e-6)
#   "f32r" — fp32 storage, ~TF32 multiply precision, 1 col/cycle on PE
#   "f16"  — fp16 storage (half the DMA bytes), 1 col/cycle, eps 4.9e-4
MM_MODE = "f16"

B, A, D, S = 2048, 64, 384, 4
H1, H2, H3 = 128, 128, 64
NCORES = 8
BC = B // NCORES  # 256 batch per core
NCH = D // 128  # 3 k-chunks for layer 0
DMA_GROUP = 1  # units per x DMA

# Set by test harness to collect a profile; kernel() stores exec_time_ns here.
PROFILE = False
LAST_EXEC_NS = None
LAST_RESULTS = None

# ---------------------------------------------------------------------------
# Walrus in this toolchain rejects >1 sync-wait per instruction ("Too many
# sync wait commands", CoreV3GenImpl setupSyncWait).  Tile's semaphore
# assignment freely attaches several waits to one instruction, so any real
# Tile kernel trips it.  Post-pass: hoist all but one wait onto injected
# NoOps on the same engine queue immediately before the instruction — the
# queue executes them in order, so the blocking semantics are identical.
# ---------------------------------------------------------------------------


def _split_multi_waits(nc):
    import bass_rust

    n_split = 0
    for blk in nc.main_func.blocks:
        insts = blk.instructions
        idx = 0
        while idx < len(insts):
            ins = insts[idx]
            si = ins.sync_info
            if si is not None and si.on_wait and len(si.on_wait) > 1:
                waits = list(si.on_wait)
                si.on_wait = [waits[-1]]
                for w in waits[:-1]:
                    nop = bass_rust.InstNoOp(
                        name=nc.get_next_instruction_name(), ins=[], outs=[]
                    )
                    nop.engine = ins.engine
                    nop.sync_info = mybir.SyncInfo(on_wait=[w], on_update=[])
                    nc.register_instruction(nop)
                    insts.insert(idx, nop)
                    idx += 1
                    n_split += 1
            idx += 1
    return n_split


def _build_units(species: np.ndarray):
    """Group atom indices by species into blocks of 4 / 2 / 1 atoms.

    Quads (4 atoms, one species) take the fast path: merged [128, 1024]
    activations, col-tiled layer 2 (both halves on 128 partitions) and a
    stacked-W3 layer 3.  Remainders (<=3 atoms per species) run as pairs /
    singles; smallest blocks go last to shorten the final drain chain.
    """
    quads, pairs, singles = [], [], []
    for s in range(S):
        atoms = [int(a) for a in np.nonzero(species == s)[0]]
        i = 0
        while len(atoms) - i >= 4:
            quads.append((s, atoms[i : i + 4]))
            i += 4
        if len(atoms) - i >= 2:
            pairs.append((s, atoms[i : i + 2]))
            i += 2
        if len(atoms) - i:
            singles.append((s, atoms[i:]))
    return quads + pairs + singles


# Weight blob column layout (one [128, WCOLS] SBUF tile / DRAM tensor).
OFF_W0 = 0  # [s][c] at OFF_W0 + (s*NCH + c)*128, 128 cols, 128 parts
OFF_W1 = OFF_W0 + S * NCH * 128  # [s] at OFF_W1 + s*128, 128 cols
OFF_W2 = OFF_W1 + S * 128  # [s] at OFF_W2 + s*64, 64 cols
OFF_W3 = OFF_W2 + S * 64  # [s] at OFF_W3 + s, 1 col, 64 parts
OFF_W3Q = OFF_W3 + S  # [s]: W3[s] stacked twice, 1 col, 128 parts
OFF_B0 = OFF_W3Q + S  # [s] at OFF_B0 + s, 1 col
OFF_B1 = OFF_B0 + S
OFF_B2 = OFF_B1 + S  # 64 parts
OFF_B2Q = OFF_B2 + S  # b2[s] stacked twice, 128 parts
WCOLS = OFF_B2Q + S


def _pack_weights(W0, b0, W1, b1, W2, b2, W3, b3):
    c = math.sqrt(math.pi) / 2.0  # undo Derivative_Erf's 2/sqrt(pi)
    dt = np.float16 if MM_MODE == "f16" else np.float32
    blob = np.zeros((128, WCOLS), dt)
    for s in range(S):
        for ch in range(NCH):
            blob[:, OFF_W0 + (s * NCH + ch) * 128 : OFF_W0 + (s * NCH + ch + 1) * 128] = (
                W0[s, ch * 128 : (ch + 1) * 128, :]
            )
        blob[:, OFF_W1 + s * 128 : OFF_W1 + (s + 1) * 128] = W1[s] * c
        blob[:, OFF_W2 + s * 64 : OFF_W2 + (s + 1) * 64] = W2[s] * c
        blob[:H3, OFF_W3 + s] = W3[s][:, 0] * c
        blob[:H3, OFF_W3Q + s] = W3[s][:, 0] * c
        blob[H3:, OFF_W3Q + s] = W3[s][:, 0] * c
        blob[:, OFF_B0 + s] = b0[s]
        blob[:, OFF_B1 + s] = b1[s]
        blob[:H3, OFF_B2 + s] = b2[s]
        blob[:H3, OFF_B2Q + s] = b2[s]
        blob[H3:, OFF_B2Q + s] = b2[s]
    return blob


def _pack_x(x, units):
    """Per-core flat x arrays.

    Per unit: block [128, NCH * w] where w = 256 * n_atoms; within chunk c the
    columns are (atom, b) so each layer-0 matmul rhs is [:, c*w:(c+1)*w].
    Returns (flat arrays per core, unit column offsets (in flat elems)).
    """
    # [A, D, B] so per (atom, chunk) the [128, BC] block is contiguous-ish
    dt = np.float16 if MM_MODE == "f16" else np.float32
    xt = np.ascontiguousarray(x.transpose(1, 2, 0).astype(dt))  # [A, D, B]
    total = sum(128 * NCH * 256 * len(a) for _, a in units)
    per_core = []
    for core in range(NCORES):
        bsl = slice(core * BC, (core + 1) * BC)
        groups = []
        for gi in range(0, len(units), DMA_GROUP):
            # one [128, gcols] C-order block per DMA group
            cols = []
            for s, atoms in units[gi : gi + DMA_GROUP]:
                # [n_a, D, BC] -> [n_a, NCH, 128, BC] -> [128, NCH, n_a, BC]
                blk = xt[atoms, :, bsl].reshape(len(atoms), NCH, 128, BC)
                cols.append(blk.transpose(2, 1, 0, 3).reshape(128, -1))
            groups.append(np.hstack(cols).reshape(-1))
        flat = np.concatenate(groups)
        assert flat.size == total
        per_core.append(flat)
    return per_core, None, total


def _build_program(units, total_x, repeat=1):
    MDT = {"f32": F32, "f32r": F32R, "f16": F16}[MM_MODE]
    nc = bass.Bass()
    xin = nc.dram_tensor("xin", [total_x], MDT, kind="ExternalInput")
    wts = nc.dram_tensor("wts", [128, WCOLS], MDT, kind="ExternalInput")
    xout = nc.dram_tensor("xout", [1, 512], F32, kind="ExternalOutput")

    with tile.TileContext(nc) as tc:
        with (
            tc.tile_pool(name="wpool", bufs=1) as wpool,
            tc.tile_pool(name="xpool", bufs=6) as xpool,
            tc.tile_pool(name="apool", bufs=3) as apool,
            tc.tile_pool(name="opool", bufs=1) as opool,
            tc.tile_pool(name="y0pool", bufs=2, space="PSUM") as y0pool,
            tc.tile_pool(name="y1pool", bufs=1, space="PSUM") as y1pool,
            tc.tile_pool(name="y2pool", bufs=1, space="PSUM") as y2pool,
            tc.tile_pool(name="outp", bufs=1, space="PSUM") as outp,
        ):
            def body():
                wt = wpool.tile([128, WCOLS], MDT, tag="wt")
                nc.sync.dma_start(wt[:], wts[:])

                out_ps = outp.tile([1, 512], F32, tag="ops")

                def w0_ap(s, ch):
                    o = OFF_W0 + (s * NCH + ch) * 128
                    return wt[:, o : o + 128]

                def bias_ap(off, s, p=128):
                    ap = wt[:p, off + s : off + s + 1]
                    return ap.bitcast(F32) if MDT is F32R else ap

                # DMA x in groups of DMA_GROUP units (bigger transfers)
                uoff = 0
                xtiles = {}
                for gi in range(0, len(units), DMA_GROUP):
                    grp = units[gi : gi + DMA_GROUP]
                    gcols = sum(NCH * 256 * len(a) for _, a in grp)
                    gt = xpool.tile([128, gcols], MDT, tag="xu")
                    nc.sync.dma_start(
                        gt[:, :gcols],
                        xin[uoff : uoff + 128 * gcols].rearrange(
                            "(p n) -> p n", p=128
                        ),
                    )
                    uoff += 128 * gcols
                    co = 0
                    for j, (s, atoms) in enumerate(grp):
                        ucols = NCH * 256 * len(atoms)
                        xtiles[gi + j] = gt[:, co : co + ucols]
                        co += ucols

                for ui, (s, atoms) in enumerate(units):
                    w = 256 * len(atoms)
                    xt_u = xtiles[ui]
                    first, last = ui == 0, ui == len(units) - 1
                    w1_ap = wt[:, OFF_W1 + s * 128 : OFF_W1 + (s + 1) * 128]
                    w2_ap = wt[:, OFF_W2 + s * 64 : OFF_W2 + (s + 1) * 64]

                    if len(atoms) == 4:
                        # ---- quad fast path, w = 1024 (2 PSUM banks L0/L1)
                        y0 = y0pool.tile([128, 1024], F32, tag="y0")
                        for ch in range(NCH):
                            for h in range(2):
                                nc.tensor.matmul(
                                    y0[:, h * 512 : (h + 1) * 512],
                                    w0_ap(s, ch),
                                    xt_u[:, ch * w + h * 512 : ch * w + (h + 1) * 512],
                                    start=(ch == 0),
                                    stop=(ch == NCH - 1),
                                )
                        a0 = apool.tile([128, 1024], MDT, tag="a0")
                        nc.scalar.activation(
                            a0[:], y0[:], AF.Derivative_Erf,
                            bias=bias_ap(OFF_B0, s),
                        )
                        y1 = y1pool.tile([128, 1024], F32, tag="y1")
                        for h in range(2):
                            nc.tensor.matmul(
                                y1[:, h * 512 : (h + 1) * 512], w1_ap,
                                a0[:, h * 512 : (h + 1) * 512],
                                start=True, stop=True,
                            )
                        a1 = apool.tile([128, 1024], MDT, tag="a1")
                        nc.scalar.activation(
                            a1[:], y1[:], AF.Derivative_Erf,
                            bias=bias_ap(OFF_B1, s),
                        )
                        # ---- layer 2 col-tiled: halves stacked on partitions
                        y2 = y2pool.tile([128, 512], F32, tag="y2")
                        nc.tensor.matmul(
                            y2[0:64, :], w2_ap, a1[:, 0:512],
                            start=True, stop=True,
                        )
                        nc.tensor.matmul(
                            y2[64:128, :], w2_ap, a1[:, 512:1024],
                            start=True, stop=True, tile_position=(0, 64),
                        )
                        a2 = apool.tile([128, 512], MDT, tag="a2")
                        nc.scalar.activation(
                            a2[:], y2[:], AF.Derivative_Erf,
                            bias=bias_ap(OFF_B2Q, s),
                        )
                        # ---- layer 3: stacked W3, contracts both halves
                        nc.tensor.matmul(
                            out_ps[:, :512],
                            wt[:, OFF_W3Q + s : OFF_W3Q + s + 1],
                            a2[:],
                            start=first, stop=last,
                        )
                        continue

                    # ---- pair / single path (w = 512 / 256)
                    y0 = y0pool.tile([128, 512], F32, tag="y0")
                    for ch in range(NCH):
                        nc.tensor.matmul(
                            y0[:, :w],
                            w0_ap(s, ch),
                            xt_u[:, ch * w : (ch + 1) * w],
                            start=(ch == 0),
                            stop=(ch == NCH - 1),
                        )
                    a0 = apool.tile([128, 512], MDT, tag="a0")
                    nc.scalar.activation(
                        a0[:, :w], y0[:, :w], AF.Derivative_Erf,
                        bias=bias_ap(OFF_B0, s),
                    )

                    y1 = y1pool.tile([128, 512], F32, tag="y1")
                    nc.tensor.matmul(
                        y1[:, :w], w1_ap, a0[:, :w], start=True, stop=True,
                    )
                    a1 = apool.tile([128, 512], MDT, tag="a1")
                    nc.scalar.activation(
                        a1[:, :w], y1[:, :w], AF.Derivative_Erf,
                        bias=bias_ap(OFF_B1, s),
                    )

                    y2 = y2pool.tile([64, 512], F32, tag="y2")
                    nc.tensor.matmul(
                        y2[:, :w], w2_ap, a1[:, :w], start=True, stop=True,
                    )
                    a2 = apool.tile([64, 512], MDT, tag="a2")
                    nc.scalar.activation(
                        a2[:, :w], y2[:, :w], AF.Derivative_Erf,
                        bias=bias_ap(OFF_B2, s, p=H3),
                    )

                    nc.tensor.matmul(
                        out_ps[:, :w],
                        wt[:H3, OFF_W3 + s : OFF_W3 + s + 1],
                        a2[:, :w],
                        start=first, stop=last,
                    )

                ot = opool.tile([1, 512], F32, tag="ot")
                nc.vector.tensor_copy(ot[:], out_ps[:])
                nc.sync.dma_start(xout[:], ot[:])

            if repeat == 1:
                body()
            else:
                with tc.For_i(0, repeat, 1):
                    body()

    _split_multi_waits(nc)
    return nc


def _prep(x, species, W0, b0, W1, b1, W2, b2, W3, b3):
    x = np.asarray(x, np.float32)
    species = np.asarray(species)
    units = _build_units(species)
    blob = _pack_weights(
        np.asarray(W0, np.float32), np.asarray(b0, np.float32),
        np.asarray(W1, np.float32), np.asarray(b1, np.float32),
        np.asarray(W2, np.float32), np.asarray(b2, np.float32),
        np.asarray(W3, np.float32), np.asarray(b3, np.float32),
    )
    xs, _, total = _pack_x(x, units)
    b3sum = float(np.asarray(b3, np.float64)[species, 0].sum())
    in_maps = [{"xin": xs[c], "wts": blob} for c in range(NCORES)]
    return units, total, in_maps, b3sum


def kernel(x, species, W0, b0, W1, b1, W2, b2, W3, b3):
    global LAST_EXEC_NS, LAST_RESULTS
    units, total, in_maps, b3sum = _prep(
        x, species, W0, b0, W1, b1, W2, b2, W3, b3
    )
    nc = _build_program(units, total)
    res = run_bass_kernel_spmd(nc, in_maps, list(range(NCORES)))
    LAST_EXEC_NS = res.exec_time_ns
    LAST_RESULTS = res
    out = np.empty(B, np.float32)
    for c in range(NCORES):
        v = res.results[c]["xout"].reshape(512)
        out[c * BC : (c + 1) * BC] = (
            v[:256].astype(np.float64) + v[256:].astype(np.float64) + b3sum
        ).astype(np.float32)
    return out


def bench(x, species, W0, b0, W1, b1, W2, b2, W3, b3,
          reps=(256, 32768), tries=5):
    """Per-invocation HW time via on-device For_i loop slope.

    Runs the kernel body R times inside one NEFF for each R in reps and
    wall-clocks the execute call; the slope between the two R values
    cancels tunnel/upload overhead.  Includes ~2-3us/iter of Tile loop
    back-edge barrier cost (constant across kernel versions).
    """
    import time as _time

    units, total, in_maps, _ = _prep(
        x, species, W0, b0, W1, b1, W2, b2, W3, b3
    )
    cores = list(range(NCORES))
    timings = {}
    for R in reps:
        nc = _build_program(units, total, repeat=R)
        ts = []
        for _ in range(tries):
            t0 = _time.perf_counter()
            run_bass_kernel_spmd(nc, in_maps, cores)
            ts.append(_time.perf_counter() - t0)
        timings[R] = min(ts[1:]) if len(ts) > 1 else ts[0]
    r0, r1 = min(reps), max(reps)
    ns = (timings[r1] - timings[r0]) / (r1 - r0) * 1e9
    return ns, timings
